# revision 33
# baseline (speedup 1.0000x reference)
"""BoundaryLoss Trainium2 kernel (V23): merged-tail layout, split chunks.

~41.5-42.7us (vs 87.6us baseline, 2.1x).  Tail merge: the four per-image
tail windows (16 valid lanes each but a full 512-column free dim) are
packed into ONE shared slab -- all four images' last 16 rows live at
partitions 16j+k, driven by a block-diagonal 5-tap band (input rows
20j+k..+4).  That removes 4x512 free-dim columns from every
exp/ln/product pass, 3 of 4 tail mask units, 6 matmuls, and ~0.5MB of
junk DMA.  The ln+masked-product accumulation runs in 7 chunks (tail
first, images 0 and 3 split in half) so the DVE product pipeline starts
as early as possible and drains right behind the last ln.

Core design:
- per image ONE packed fp8 tensor [128, 4, 516+512]: per 128-row window,
  516 bytes of zero-padded target as 0/1 INTEGER bytes (fp8 denormal
  coding: byte k == k*2^-9 exactly), then 512 bytes of PRESIGNED pred
  y = (1-2t)*x (bce = softplus(y); invalid lanes y=-240 -> softplus 0).
- v = t0+t2, v3 = v+t4 as u16 packed byte adds (even shifts, no carry),
  box sum s = Band.T @ v3[c] + Band.T @ v[c+1]: 2 fp8 matmuls/window.
- bce via ACT Exp then Ln(1+ey) with accum_out; phases dep-forced apart
  (2 table loads).  Non-boundary mask Q=(s-12.5*2^-9)^2 > (12*2^-9)^2:
  window pair (0,1) via ACT Square from PSUM, pair (2,3) + tail via DVE
  TT add + TT mult.  One stt product per accumulation chunk.
- all DMAs on the sync HWDGE ring, 128-row outer dims, tail slab and a
  1-window first chunk land first so ACT starts early.
"""

import numpy as np

import concourse.bass as bass
import concourse.bacc as bacc_mod
import concourse.tile as tile
from concourse import mybir
from concourse.bass_utils import run_bass_kernel_spmd

F32 = mybir.dt.float32
BF16 = mybir.dt.bfloat16
FP8 = mybir.dt.float8e4
U16 = mybir.dt.uint16
ALU = mybir.AluOpType
ACTF = mybir.ActivationFunctionType

B, H, W = 32, 512, 512
NCORES = 8
IMGS = B // NCORES          # 4 images per core
PAD = 2
TP = H + 2 * PAD            # 516
NWIN = 4                    # main 124-row windows per image
PKC = TP + W                # 1028 packed bytes per (partition, window)
SC = 2.0 ** -9              # denormal coding scale of the 0/1 target bytes
QTHR = 144.0 * SC * SC      # (s-12.5)^2 > 144  <=>  s in {0, 25}

# accumulation chunks: tail, img0 split, imgs 1-2 whole, img3 split
NCH = 7
NSTAT = 2 * NCH


def _ap3(t, off, dims):
    return bass.AP(t, off, dims)


def _build_nc() -> bass.Bass:
    nc = bacc_mod.Bacc(trn_type="TRN2")

    pkm = nc.dram_tensor("pkm", [IMGS, 128, NWIN, PKC], FP8, kind="ExternalInput")
    pkt = nc.dram_tensor("pkt", [128, PKC], FP8, kind="ExternalInput")
    bands = nc.dram_tensor("bands", [128, 2, 128], FP8, kind="ExternalInput")
    stats = nc.dram_tensor("stats", [128, NSTAT], F32, kind="ExternalOutput")

    with tile.TileContext(nc) as tc:
        with (
            tc.tile_pool(name="singles", bufs=1) as singles,
            tc.tile_pool(name="pkin", bufs=4) as pkin,
            tc.tile_pool(name="vp", bufs=4) as vp,
            tc.tile_pool(name="v3p", bufs=4) as v3p,
            tc.tile_pool(name="eyp", bufs=4) as eyp,
            tc.tile_pool(name="qmp", bufs=4) as qmp,
            tc.tile_pool(name="spp", bufs=4) as spp,
            tc.tile_pool(name="dp", bufs=6) as dp,
            tc.tile_pool(name="scrp", bufs=4) as scrp,
            tc.tile_pool(name="ps2", bufs=3, space="PSUM") as ps2,
            tc.tile_pool(name="ps1", bufs=2, space="PSUM") as ps1,
        ):
            pk_sb = [None] * IMGS
            v_sb = [None] * IMGS
            v3_sb = [None] * IMGS
            ey_sb = [None] * IMGS
            q_sb = [None] * IMGS

            # ---- input DMAs on the sync ring: tail slab first (small),
            # then image 0's first window, the rest, band mid-queue.
            pkt_sb = singles.tile([128, PKC], FP8)
            nc.sync.dma_start(pkt_sb[:], pkt[:, :])
            band_sb = singles.tile([128, 2, 128], FP8)
            for i in range(IMGS):
                pk_sb[i] = pkin.tile([128, NWIN, PKC], FP8, tag="pk",
                                     name=f"pk{i}")
                if i == 0:
                    nc.sync.dma_start(
                        pk_sb[i][:, 0:2, :],
                        _ap3(pkm, 0, [[NWIN * PKC, 128], [PKC, 2], [1, PKC]]),
                    )
                    nc.sync.dma_start(
                        pk_sb[i][:, 2:NWIN, :],
                        _ap3(pkm, 2 * PKC,
                             [[NWIN * PKC, 128], [PKC, NWIN - 2], [1, PKC]]),
                    )
                else:
                    nc.sync.dma_start(
                        pk_sb[i][:],
                        _ap3(pkm, i * 128 * NWIN * PKC,
                             [[NWIN * PKC, 128], [PKC, NWIN], [1, PKC]]),
                    )
                if i == 1:
                    nc.sync.dma_start(band_sb[:], bands[:, :, :])


            stats_sb = singles.tile([128, NSTAT], F32)
            nc.vector.memset(stats_sb[:], 0.0)
            nbias = singles.tile([128, 2, W], BF16)
            nc.vector.memset(nbias[:], -12.5 * SC)
            bias_sq = singles.tile([128, 1], F32)
            nc.gpsimd.memset(bias_sq[:], -12.5 * SC)

            exp_insts = []
            sq_insts = []

            # ---- tail slab phase 1: v/v3 (DVE), exp (ACT, first ACT op)
            vT = singles.tile([128, TP - 2], FP8)
            nc.vector.tensor_tensor(
                vT[:].bitcast(U16),
                pkt_sb[:, 0:TP - 2].bitcast(U16),
                pkt_sb[:, 2:TP].bitcast(U16),
                op=ALU.add,
            )
            v3T = singles.tile([128, W], FP8)
            nc.vector.tensor_tensor(
                v3T[:].bitcast(U16),
                vT[:, 0:W].bitcast(U16),
                pkt_sb[:, 4:4 + W].bitcast(U16),
                op=ALU.add,
            )
            eyT = singles.tile([128, W], F32)
            exp_insts.append(
                nc.scalar.activation(eyT[:], pkt_sb[:, TP:PKC], ACTF.Exp))

            # ---- per image phase 1: v, v3 (DVE), exp (ACT)
            for i in range(IMGS):
                tpk = pk_sb[i]
                v_sb[i] = vp.tile([128, NWIN, TP - 2], FP8, tag="v", name=f"v{i}")
                nc.vector.tensor_tensor(
                    v_sb[i][:].bitcast(U16),
                    tpk[:, :, 0:TP - 2].bitcast(U16),
                    tpk[:, :, 2:TP].bitcast(U16),
                    op=ALU.add,
                )
                v3_sb[i] = v3p.tile([128, NWIN, W], FP8, tag="v3", name=f"v3{i}")
                nc.vector.tensor_tensor(
                    v3_sb[i][:].bitcast(U16),
                    v_sb[i][:, :, 0:W].bitcast(U16),
                    tpk[:, :, 4:4 + W].bitcast(U16),
                    op=ALU.add,
                )
                ey_sb[i] = eyp.tile([128, NWIN, W], F32, tag="ey", name=f"ey{i}")
                if i == 0:
                    exp_insts.append(nc.scalar.activation(
                        ey_sb[i][:, 0:2, :], tpk[:, 0:2, TP:PKC], ACTF.Exp))
                    exp_insts.append(nc.scalar.activation(
                        ey_sb[i][:, 2:NWIN, :], tpk[:, 2:NWIN, TP:PKC], ACTF.Exp))
                else:
                    exp_insts.append(nc.scalar.activation(
                        ey_sb[i][:], tpk[:, :, TP:PKC], ACTF.Exp))

            # ---- tail slab box matmuls + mask precursor (DVE path)
            sT = ps1.tile([128, W], F32)
            nc.tensor.matmul(sT[:], band_sb[:, 1, :], v3T[:, 0:W],
                             start=True, stop=False)
            nc.tensor.matmul(sT[:], band_sb[:, 1, :], vT[:, 1:W + 1],
                             start=False, stop=True)
            dT = dp.tile([128, W], BF16, tag="dT")
            nc.vector.tensor_tensor(dT[:], sT[:], nbias[:, 0, :], op=ALU.add)
            qT = singles.tile([128, W], BF16)
            nc.vector.tensor_tensor(qT[:], dT[:], dT[:], op=ALU.mult)

            # ---- per image box matmuls + mask precursor Q = (s-12.5*SC)^2
            for i in range(IMGS):
                q_sb[i] = qmp.tile([128, NWIN, W], BF16, tag="q", name=f"q{i}")
                for g in range(2):          # window pairs (0,1) and (2,3)
                    s2 = ps2.tile([128, 2, W], F32, tag="s2")
                    for j in range(2):
                        w = 2 * g + j
                        nc.tensor.matmul(
                            s2[:, j, :], band_sb[:, 0, :], v3_sb[i][:, w, 0:W],
                            start=True, stop=False)
                        nc.tensor.matmul(
                            s2[:, j, :], band_sb[:, 0, :], v_sb[i][:, w, 1:W + 1],
                            start=False, stop=True)
                    if g == 0:
                        sq_inst = nc.scalar.activation(
                            q_sb[i][:, 0:2, :], s2[:], ACTF.Square,
                            bias=bias_sq[:])
                        for e in exp_insts:
                            tile.add_dep_helper(sq_inst.ins, e.ins, sync=True,
                                                reason="squares after exps")
                        sq_insts.append(sq_inst)
                    else:
                        d = dp.tile([128, 2, W], BF16, tag="d")
                        nc.vector.tensor_tensor(d[:], s2[:], nbias[:], op=ALU.add)
                        nc.vector.tensor_tensor(
                            q_sb[i][:, 2:4, :], d[:], d[:], op=ALU.mult)

            # ---- phase 2: ln (+accum) then one masked product per chunk.
            # tail chunk first (its inputs are ready earliest), image 3
            # split so its product pipeline drains sooner.
            def do_chunk(ci, ey_ap, q_ap, spy_shape):
                spy = spp.tile(spy_shape, BF16, tag="spy", name=f"spy{ci}")
                ln_inst = nc.scalar.activation(
                    spy[:], ey_ap, ACTF.Ln, bias=1.0,
                    accum_out=stats_sb[:, ci:ci + 1],
                )
                for e in exp_insts:
                    tile.add_dep_helper(ln_inst.ins, e.ins, sync=True,
                                        reason="keep exp/ln table phases apart")

                scr = scrp.tile(spy_shape, BF16, tag="scr", name=f"scr{ci}")
                nc.vector.scalar_tensor_tensor(
                    scr[:], q_ap, QTHR, spy[:],
                    op0=ALU.is_gt, op1=ALU.mult,
                    accum_out=stats_sb[:, NCH + ci:NCH + ci + 1],
                )

            do_chunk(0, eyT[:], qT[:], [128, W])
            do_chunk(1, ey_sb[0][:, 0:2, :], q_sb[0][:, 0:2, :], [128, 2, W])
            do_chunk(2, ey_sb[0][:, 2:4, :], q_sb[0][:, 2:4, :], [128, 2, W])
            do_chunk(3, ey_sb[1][:], q_sb[1][:], [128, NWIN, W])
            do_chunk(4, ey_sb[2][:], q_sb[2][:], [128, NWIN, W])
            do_chunk(5, ey_sb[3][:, 0:2, :], q_sb[3][:, 0:2, :], [128, 2, W])
            do_chunk(6, ey_sb[3][:, 2:4, :], q_sb[3][:, 2:4, :], [128, 2, W])

            nc.sync.dma_start(stats[:, :], stats_sb[:])

    nc.compile()
    nc.finalize()
    return nc


_NC = None


def _get_nc() -> bass.Bass:
    global _NC
    if _NC is None:
        _NC = _build_nc()
    return _NC


def _make_in_maps(pred: np.ndarray, target: np.ndarray) -> list[dict]:
    import ml_dtypes

    fp8 = ml_dtypes.float8_e4m3fn
    x8 = pred.reshape(B, H, W).astype(fp8)
    t_u8 = target.reshape(B, H, W).astype(np.uint8)
    ysig = (x8.view(np.uint8) ^ (t_u8 << 7))            # presigned, uint8
    junk = np.asarray(-240.0, dtype=fp8).view(np.uint8).item()   # 0xF7

    tpad = np.zeros((B, TP, TP), dtype=np.uint8)
    tpad[:, PAD:PAD + H, PAD:PAD + W] = t_u8

    # main windows: tpad rows 124w + p
    win_is = [0, 124, 248, 372]
    rows = np.asarray(win_is)[:, None] + np.arange(128)[None, :]  # [4, 128]
    twin = tpad[:, rows, :].transpose(0, 2, 1, 3)        # [B,128,4,516] u8

    ymain = np.full((B, 128, NWIN, W), junk, dtype=np.uint8)
    for g in range(NWIN):
        ymain[:, 2:126, g, :] = ysig[:, 124 * g:124 * g + 124, :]

    pkm = np.ascontiguousarray(
        np.concatenate([twin, ymain], axis=3)).view(fp8)  # [B,128,4,1028]

    # tail slab, per core: partition 20j+r = tpad row 496+r of image j;
    # y at partition 16j+k = presigned row 496+k of image j
    band_m = np.zeros((128, 128), dtype=np.float32)
    for m in range(2, 126):
        band_m[m - 2:m + 3, m] = 1.0
    band_t = np.zeros((128, 128), dtype=np.float32)
    for j in range(IMGS):
        for k in range(16):
            band_t[20 * j + k:20 * j + k + 5, 16 * j + k] = 1.0
    bands = np.stack([band_m, band_t], axis=1).astype(fp8)   # [128, 2, 128]

    in_maps = []
    for c in range(NCORES):
        sl = slice(c * IMGS, (c + 1) * IMGS)
        ttail = np.zeros((128, TP), dtype=np.uint8)
        ytail = np.full((128, W), junk, dtype=np.uint8)
        for j in range(IMGS):
            ttail[20 * j:20 * j + 20, :] = tpad[c * IMGS + j, 496:516, :]
            ytail[16 * j:16 * j + 16, :] = ysig[c * IMGS + j, 496:512, :]
        pkt = np.ascontiguousarray(
            np.concatenate([ttail, ytail], axis=1)).view(fp8)  # [128, 1028]
        in_maps.append(
            {
                "pkm": np.ascontiguousarray(pkm[sl]),
                "pkt": pkt,
                "bands": bands,
            }
        )
    return in_maps


def _finish(results: list[dict]) -> np.ndarray:
    total = 0.0
    for res in results:
        st = res["stats"].astype(np.float64)
        total += 5.0 * st[:, 0:NCH].sum()
        total -= 4.0 * st[:, NCH:].sum()
    mean = total / float(B * H * W)
    return np.asarray(np.float32(mean))


def kernel(pred: np.ndarray, target: np.ndarray, **run_kwargs) -> np.ndarray:
    pred = np.asarray(pred)
    target = np.asarray(target)
    nc = _get_nc()
    in_maps = _make_in_maps(pred, target)
    out = run_bass_kernel_spmd(nc, in_maps, core_ids=list(range(NCORES)), **run_kwargs)
    res = _finish(out.results)
    kernel.last_run = out
    return res
